# revision 47
# baseline (speedup 1.0000x reference)
"""Trainium2 Bass kernel for the additive-attention module.

Reference math (single device):
    enc    = einsum('sbh,kh->sbk', encoder_output, We) + be     # [S,B,K]
    hid    = hidden @ Wh.T + bh                                 # [B,K]
    energy = sigmoid(enc + hid[None]) @ Wv.T + bv               # [S,B,1]
    attn   = softmax(energy, axis=0)                            # over S
    out    = attn.transpose(1, 2, 0)                            # [B,1,S]

Device strategy (8 NeuronCores, data-parallel over batch):
  * Each core gets 8 of the 64 batches; weights replicated.
  * encoder_output is cast to fp8e4m3 on the host and laid out so each
    (s-block, batch) tile [128, 4, 1024] is a single contiguous 512 KB
    region in HBM - DMA streams it at full bandwidth.
  * sigmoid(x) = (1 + tanh(x/2))/2, and softmax is invariant to the
    affine constants, so the device computes
        E[s,b] = sum_k Wv[k] * tanh(0.5*enc_raw + hidb)
    (hidb = 0.5*(hidden @ Wh.T + bh + be), computed on host - 17 MFLOP)
    and the host finishes with softmax(0.5 * E) in float64.
  * enc matmuls run as fp8e4 DoubleRow; We host-scaled by 32 for fp8
    range, compensated in the activation input scale.
  * The PE is the bottleneck and runs at its structural floor: a warm
    (2.4 GHz, HAM K=8/8) fp8 DoubleRow matmul with free dim 512 costs
    exactly 512 cycles back-to-back, so 512 enc + 128 reduce matmuls =
    136.5us/core.  Everything else is engineered to keep the PE stream
    stall-free and the bookends (startup DMA, drain) off that path.
  * The tanh work (16.8M elems/core) is split across the Scalar engine
    (exact tanh) and the Vector engine (odd-quintic custom DVE op,
    u*(1 + u^2*(c3 + c5*u^2)), u = A*z, rms err 0.0045), greedily
    balanced by estimated per-tile cost.  Both write fp8 into shared
    [128, 2, 1024] sig tiles.
  * Iterations run in kc-major superblocks of 3 so each (kc, hc)
    stationary serves 6 consecutive matmuls; tile_legalize's per-matmul
    LDWEIGHTS are deduplicated post-build (see _dedupe_ldweights).
  * The Wv reduction over k rides the PE as fp8 DoubleRow with a
    2-column stationary operand (hi/lo pair: fp8(Wv*256) and its
    residual*16; the host recombines r0 + r1/16).  Reduces trail the
    enc stream by 2 superblocks so their sig semaphores are satisfied
    before the in-order PE reaches them.
  * Each iteration t directs its hi/lo result to partition rows
    (2t, 2t+1) of a persistent PSUM bank per s-half; one [64, 512]
    PSUM->SBUF copy + one scatter-DMA per s-half at the very end.
  * Startup: the PE clock idles at 1.2 GHz until ~3.4us of sustained
    activity (HAM); 28 dummy matmuls over a memset scratch burn the
    initial DMA wait so the real stream starts at 2.4 GHz.  WeT and
    iteration 0 lead the gpsimd SWDGE queue (the only ~350 GB/s path).
"""

import os
import numpy as np

import concourse.bass as bass
import concourse.mybir as mybir
import concourse.tile as tile
from concourse import bacc
from concourse.bass_utils import run_bass_kernel_spmd

S_TOT = 4096
B_TOT = 64
H = 512
N_CORES = 8
BPC = B_TOT // N_CORES  # batches per core
P = 128
KC = H // P  # 4 contraction / output chunks
SH = 1024    # s-chunk processed per activation tile
NMM = 512    # matmul moving free dim
NBLK = S_TOT // SH

F32 = mybir.dt.float32
F8 = mybir.dt.float8e4
WE_SCALE = 32.0
WV_SCALE = 256.0

# Odd-quintic tanh fit  t(z) = u*(1 + u^2*(QC3 + QC5*u^2)), u = QA*z,
# least-squares on the actual pre-activation distribution (|z| <= 2.3).
QA = 0.98079
QC3 = -0.25261
QC5 = 0.029919

# Results of the most recent device run (for the local test harness only).
LAST_RESULTS = None

_BUILD_CACHE = {}
_PWL_OP = None


def _dedupe_ldweights(nc):
    """Remove redundant InstLdweights after tile legalization.

    tile_legalize splits every InstMatmult into LDWEIGHTS + non-self-
    loading MATMUL.  The LDWEIGHTS serializes with the matmul on the PE
    (256 cycles each for a [128,2,128] fp8 DoubleRow stationary), so a
    stream of matmuls re-using one stationary pays 2x PE time for no
    reason.  A duplicate LDWEIGHTS (same operand AP as the previous one
    on the PE stream, no intervening self-loading matmul, and no
    attached semaphore waits/updates) is dead weight - delete it.
    Must run BEFORE nc.compile(), which moves matmul waits onto the
    preceding LDWEIGHTS.
    """
    import json
    for fn in nc.m.functions:
        for bb in fn.blocks:
            last_key = None
            to_remove = []
            for inst in bb.instructions:
                tn = type(inst).__name__
                if getattr(inst, "engine", None) != mybir.EngineType.PE:
                    continue
                if tn == "InstLdweights":
                    d = json.loads(mybir.instruction_to_pretty_json_string(inst))
                    for k in ("name", "sync_info", "debug", "bass_addl_debug"):
                        d.pop(k, None)
                    key = json.dumps(d, sort_keys=True)
                    si = inst.sync_info
                    clean = si is None or (
                        len(si.on_wait) == 0 and len(si.on_update) == 0
                    )
                    if clean and key == last_key:
                        to_remove.append(inst)
                    else:
                        last_key = key
                elif tn == "InstMatmult":
                    if inst.ldweights is not False:
                        last_key = None
            for inst in to_remove:
                bb.instructions.remove(inst)


def _register_dve_tanh_op():
    """Register the custom odd-quintic DVE op (idempotent).

    body: u = Src0*C1 + C0; w = u*u; out = u*(One + w*(C2 + C3*w))
    C0 = per-partition bias (A*hidb), C1 = input scale, C2 = c3,
    C3 (spilled to in1 as a [P,1] broadcast) = c5.  Exactly 8 ALU stages.
    """
    global _PWL_OP
    if _PWL_OP is not None:
        return _PWL_OP
    import concourse.dve_ops as dve_ops_mod
    from concourse.dve_ops import DveOp
    from concourse.dve_spec import (
        Spec, Src0, C0, C1, C2, C3, One, sq, lower,
        _spill_c3_to_src1, _has_src1,
    )
    from concourse.dve_uop import DveOpSpec

    name = "TANH_QUINTIC_ANT"
    for op in dve_ops_mod.OPS:
        if op.name == name:
            _PWL_OP = op
            return op

    u = Src0 * C1 + C0
    w = sq(u)
    body = _spill_c3_to_src1(u * (One + w * (C2 + C3 * w)))

    def _ref(in0, in1, s0, s1, imm2):
        uu = in0.astype(np.float32) * s1 + s0
        ww = uu * uu
        return uu * (1.0 + ww * (imm2 + in1 * ww))

    spec = Spec(body=body, reference=_ref)
    opcode = dve_ops_mod._CUSTOM_DVE_ROW_BASE + len(dve_ops_mod.OPS)
    shas = {}
    for ver in ("v3", "v4"):
        tmp = DveOpSpec(
            name=name, opcode=opcode, uops=lower(spec, ver=ver),
            rd1_en=_has_src1(spec),
        )
        shas[ver] = tmp.sha(ver)
    op = DveOp(name, spec, subdim=False, uops_sha=shas)
    dve_ops_mod.OPS.append(op)
    dve_ops_mod._SUB_OPCODE_FOR_NAME[name] = opcode
    dve_ops_mod.CUSTOM_DVE_SPECS[name] = spec
    _PWL_OP = op
    return op


def _build(s_tot=S_TOT, bpc=BPC, n_cores=N_CORES):
    key = (s_tot, bpc, n_cores)
    if key in _BUILD_CACHE:
        return _BUILD_CACHE[key]
    pwl_op = _register_dve_tanh_op()

    nc = bacc.Bacc(
        "TRN2", target_bir_lowering=False, debug=False, num_devices=n_cores
    )
    nblk = s_tot // SH
    eo8 = nc.dram_tensor("eo8", [nblk, bpc, 2, P, KC, NMM], F8,
                         kind="ExternalInput")
    WeT = nc.dram_tensor("WeT", [P, KC, H], F8, kind="ExternalInput")
    # bias tensor, two variants per kc: [:, :, 0] = hidb (ACT tanh bias),
    # [:, :, 1] = QA*hidb (DVE quintic bias)
    hidb = nc.dram_tensor("hidb", [P, KC, 2, bpc], F32, kind="ExternalInput")
    Wvp = nc.dram_tensor("Wvp", [P, nblk * bpc, 2, 2, 64], F8,
                         kind="ExternalInput")
    c5t = nc.dram_tensor("c5t", [P, 1], F32, kind="ExternalInput")
    out = nc.dram_tensor("out", [bpc, 2, s_tot], F32, kind="ExternalOutput")
    niter = nblk * bpc

    nns = SH // NMM
    Tanh = mybir.ActivationFunctionType.Tanh
    DRm = mybir.MatmulPerfMode.DoubleRow
    # Superblock: T_SB iterations processed kc-major so each (kc, hc)
    # stationary serves T_SB*nns consecutive matmuls -> the dedupe pass
    # keeps one LDWEIGHTS per group (the LDW->matmul weight-load latency
    # bubble, ~150-200ns, then amortizes over 6 matmuls instead of 2).
    T_SB = 3
    # Reduces trail by LAG_SB superblocks so their sig semaphores are
    # satisfied before the in-order PE reaches them.
    LAG_SB = 2

    with tile.TileContext(nc) as tc:
        with (
            tc.tile_pool(name="weights", bufs=1) as wpool,
            tc.tile_pool(name="ebuf", bufs=32) as epool,
            tc.tile_pool(name="sig", bufs=40) as sigpool,
            tc.tile_pool(name="estage", bufs=4) as stpool,
            tc.tile_pool(name="enc", bufs=3, space="PSUM") as encpool,
            tc.tile_pool(name="epsum", bufs=2, space="PSUM") as enpool,
        ):
            # Startup: iteration 0's two halves ride the sync and scalar
            # HWDGE queues (parallel to everything); WeT leads the gpsimd
            # SWDGE queue (16-engine fan-out) followed by the eo stream.
            # WeT then iteration 0's halves lead the gpsimd SWDGE queue —
            # the only queue with full 16-engine fan-out (~350 GB/s once
            # ramped; the sync/scalar HWDGE queues crawl at ~25 GB/s).
            # WeT and iteration 0 are split into hc-pair pieces ordered so
            # the first matmul group (j=0, hc=0) gates on just 256 KB
            WeT_sb = wpool.tile([P, KC, H], F8, tag="WeT")
            ebufs = {}  # t -> [half0, half1]
            ebufs[0] = [epool.tile([P, KC, NMM], F8, tag="ebuf", name=f"eb0_{h}")
                        for h in range(2)]
            for hc in range(0, KC, 2):
                nc.gpsimd.dma_start(WeT_sb[:, hc:hc + 2, :],
                                    WeT.ap()[:, hc:hc + 2, :])
                for h in range(2):
                    nc.gpsimd.dma_start(ebufs[0][h][:, hc:hc + 2, :],
                                        eo8.ap()[0, 0, h, :, hc:hc + 2, :])
            hidb_sb = wpool.tile([P, KC, 2, bpc], F32, tag="hidb")
            nc.sync.dma_start(hidb_sb[:], hidb.ap())
            c5_sb = wpool.tile([P, 1], F32, tag="c5t")
            nc.sync.dma_start(c5_sb[:], c5t.ap())
            # Wvp (1 MB): first use is the first reduce (~LAG_SB superblocks
            # in); rides the fast SWDGE queue after iteration 2's triggers
            Wv_sb = wpool.tile([P, niter, 2, 2, 64], F8, tag="Wvp")

            # persistent hi/lo energy accumulators: one PSUM bank per
            # s-half; iteration t lands on partition rows (2t, 2t+1)
            eps = [
                enpool.tile([P, NMM], F32, tag="epsum", name=f"eps{ns}")
                for ns in range(nns)
            ]

            # HAM warm-up: the PE clock idles at 1.2 GHz and only doubles
            # after ~3.4us of sustained activity.  The first real matmul
            # can't start until WeT + iteration 0 land (~15us), so burn the
            # DMA wait on dependency-free dummy matmuls over a memset
            # scratch tile.  They write (start=True groups) into eps, which
            # iteration 0's real reduce erases with its own start=True.
            warm = wpool.tile([P, 2, NMM], F8, tag="warm")
            nc.vector.memset(warm[:], 0)
            for w in range(20):
                nc.tensor.matmul(
                    eps[w % 2][0:64, :], warm[:, :, 0:64], warm[:],
                    start=True, stop=True, perf_mode=DRm,
                )

            def emit_reduce(t, sigs, order=(0, 1)):
                for idx, j in enumerate(order):
                    for ns in range(nns):
                        nc.tensor.matmul(
                            eps[ns][0:64, :],
                            Wv_sb[:, t, j],
                            sigs[j][:, :, ns * NMM:(ns + 1) * NMM],
                            start=(t == 0 and idx == 0),
                            stop=(t == niter - 1 and idx == 1),
                            perf_mode=DRm,
                        )

            # greedy ACT/DVE balance by estimated per-tile cost (us)
            ACT_COST, DVE_COST = 1.125, 1.28
            eng_load = [0.0, 0.0]  # ACT, DVE

            def emit_act(sig, r, enc_t, kc, b):
                use_act = (eng_load[0] + ACT_COST) <= (eng_load[1] + DVE_COST)
                if use_act:
                    eng_load[0] += ACT_COST
                    nc.scalar.activation(
                        sig[:, r, :], enc_t[:], Tanh,
                        scale=0.5 / WE_SCALE,
                        bias=hidb_sb[:, kc, 0, b:b + 1],
                    )
                else:
                    eng_load[1] += DVE_COST
                    nc.vector._custom_dve(
                        pwl_op,
                        out=sig[:, r, :], in0=enc_t[:],
                        in1=c5_sb[:],
                        s0=hidb_sb[:, kc, 1, b:b + 1],
                        s1=QA * 0.5 / WE_SCALE,
                        imm2=QC3,
                    )

            # ramp-in: tiny first superblocks so the first matmuls only
            # gate on iteration 0's DMA, not three iterations' worth
            sbs = [[0], [1], [2]] + [
                list(range(t0, min(t0 + T_SB, niter)))
                for t0 in range(3, niter, T_SB)
            ]
            pend = []  # [(t, sigs, order), ...] awaiting reduce
            for sbi, ts in enumerate(sbs):
                # DMA triggers for this superblock (iteration 0 pre-issued)
                for t in ts:
                    if t == 0:
                        continue
                    blk, b = t // bpc, t % bpc
                    ebufs[t] = [
                        epool.tile([P, KC, NMM], F8, tag="ebuf",
                                   name=f"eb{t}_{h}")
                        for h in range(2)
                    ]
                    for h in range(2):
                        nc.gpsimd.dma_start(ebufs[t][h][:], eo8.ap()[blk, b, h])
                    if t == 2:
                        nc.gpsimd.dma_start(Wv_sb[:], Wvp.ap())
                due = len(sbs[sbi - LAG_SB]) if sbi >= LAG_SB else 0
                sigs_of = {t: {} for t in ts}
                if sbi == 0:
                    # iteration 0, DMA-paced: hc-major so the first 4
                    # matmuls only need the hc 0-1 pieces of WeT and eb0
                    t = ts[0]
                    for j in (0, 1):
                        encs = {
                            r: encpool.tile([P, SH], F32, tag="enc",
                                            name=f"enc{t}_{2 * j + r}")
                            for r in range(2)
                        }
                        sigs_of[t][j] = sigpool.tile(
                            [P, 2, SH], F8, tag="sig", name=f"sig{t}_{j}"
                        )
                        for hc in range(0, KC, 2):
                            for r in range(2):
                                kc = 2 * j + r
                                for ns in range(nns):
                                    s0 = ns * NMM
                                    nc.tensor.matmul(
                                        encs[r][:, s0:s0 + NMM],
                                        WeT_sb[:, hc:hc + 2, kc * P:(kc + 1) * P],
                                        ebufs[t][ns][:, hc:hc + 2, :],
                                        start=(hc == 0),
                                        stop=(hc == KC - 2),
                                        perf_mode=DRm,
                                    )
                        for r in range(2):
                            emit_act(sigs_of[t][j], r, encs[r], 2 * j + r,
                                     t % bpc)
                else:
                    # kc-major enc matmuls + activations as tiles complete;
                    # trailing reduces ride mid-superblock (after the j=0
                    # phases) so stationary switches stay spread out
                    for j in (0, 1):
                        for r in range(2):
                            kc = 2 * j + r
                            encs = {}
                            for t in ts:
                                encs[t] = encpool.tile(
                                    [P, SH], F32, tag="enc", name=f"enc{t}_{kc}"
                                )
                            for hc in range(0, KC, 2):
                                for t in ts:
                                    for ns in range(nns):
                                        s0 = ns * NMM
                                        nc.tensor.matmul(
                                            encs[t][:, s0:s0 + NMM],
                                            WeT_sb[:, hc:hc + 2, kc * P:(kc + 1) * P],
                                            ebufs[t][ns][:, hc:hc + 2, :],
                                            start=(hc == 0),
                                            stop=(hc == KC - 2),
                                            perf_mode=DRm,
                                        )
                            for t in ts:
                                b = t % bpc
                                if r == 0:
                                    sigs_of[t][j] = sigpool.tile(
                                        [P, 2, SH], F8, tag="sig",
                                        name=f"sig{t}_{j}"
                                    )
                                emit_act(sigs_of[t][j], r, encs[t], kc, b)
                for _ in range(due):
                    emit_reduce(*pend.pop(0))
                for t in ts:
                    order = (1, 0) if t == niter - 1 else (0, 1)
                    pend.append((t, sigs_of[t], order))
                    del ebufs[t]
            for p in pend:
                emit_reduce(*p)

            # drain: both s-halves assemble into one [64, 2, 512] stage so
            # each dram run is 4 KB (ns0|ns1 contiguous per (blk,b,h)) and
            # each queue moves 32 descriptors instead of 64 - the HWDGE
            # queues are descriptor-rate bound (~80 ns/descriptor), so
            # fewer, larger descriptors halve the exposed drain time.
            # stage row 16*blk + 2*b + h <- eps[ns][2*(blk*bpc+b)+h, :]
            out_b = out.ap().rearrange("b h (blk r) -> blk b h r", blk=nblk)
            stage = stpool.tile([2 * niter, nns, NMM], F32, tag="estage")
            nc.vector.tensor_copy(stage[:, 0, :], eps[0][0:2 * niter, :])
            nc.scalar.copy(stage[:, 1, :], eps[1][0:2 * niter, :])
            nc.sync.dma_start(out_b[0:2], stage[0:32])
            nc.scalar.dma_start(out_b[2:4], stage[32:64])

    _dedupe_ldweights(nc)
    nc.compile()
    _BUILD_CACHE[key] = nc
    return nc


def make_in_maps(hidden, encoder_output, We, be, Wh, bh, Wv):
    """Host-side sharding/layout prep. Returns per-core input dicts."""
    import ml_dtypes
    f8 = ml_dtypes.float8_e4m3fn
    eo = np.asarray(encoder_output, dtype=np.float32)
    hidden = np.asarray(hidden, dtype=np.float32)
    WeT = np.ascontiguousarray(
        (np.asarray(We, np.float32).T * WE_SCALE)
        .reshape(KC, P, H).transpose(1, 0, 2)
    ).astype(f8)  # [P, KC(hc), H(k)]

    # hidb = 0.5 * (hidden @ Wh.T + bh + be); kc 2,3 pre-scaled by QA
    hid_all = 0.5 * (
        hidden @ np.asarray(Wh, np.float32).T
        + np.asarray(bh, np.float32) + np.asarray(be, np.float32)
    )  # [B_TOT, H]

    # Wv stationary pairs for fp8 DoubleRow: [P, pair j, plane r, col]
    # col 0 = fp8(Wv*256) (hi), col 1 = fp8(16*(Wv*256 - hi)) (lo)
    wv = np.asarray(Wv, np.float32).reshape(-1) * WV_SCALE  # [H]
    wv_hi = wv.astype(f8).astype(np.float32)
    wv_lo = (wv - wv_hi) * 16.0
    niter = NBLK * BPC
    Wvp = np.zeros((P, niter, 2, 2, 64), np.float32)
    for t in range(niter):
        c = 2 * t  # column pair selects the PSUM output row pair
        for j in range(2):
            for r in range(2):
                kc = 2 * j + r
                Wvp[:, t, j, r, c] = wv_hi[kc * P:(kc + 1) * P]
                Wvp[:, t, j, r, c + 1] = wv_lo[kc * P:(kc + 1) * P]
    Wvp = Wvp.astype(f8)

    c5t = np.full((P, 1), QC5, np.float32)

    # eo8[b][blk, h, p, c, s] = eo[blk*SH + h*NMM + s, b, c*128 + p] as fp8
    eo_r = eo.reshape(NBLK, 2, NMM, B_TOT, KC, P).transpose(3, 0, 1, 5, 4, 2)
    eo8_all = np.ascontiguousarray(eo_r).astype(f8)  # [B, nblk, 2, P, KC, NMM]

    in_maps = []
    for c in range(N_CORES):
        b0 = c * BPC
        eo8_c = np.ascontiguousarray(
            eo8_all[b0:b0 + BPC].transpose(1, 0, 2, 3, 4, 5)
        )  # [nblk, BPC, 2, P, KC, NMM]
        hidb_c = hid_all[b0:b0 + BPC].T.reshape(KC, P, BPC).transpose(1, 0, 2)
        hidb_c = np.stack([hidb_c, QA * hidb_c], axis=2)  # [P, KC, 2, BPC]
        in_maps.append({
            "eo8": eo8_c,
            "WeT": WeT,
            "hidb": np.ascontiguousarray(hidb_c),
            "Wvp": Wvp,
            "c5t": c5t,
        })
    return in_maps


def kernel(hidden, encoder_output, each_size=None, We=None, be=None,
           Wh=None, bh=None, Wv=None, bv=None):
    global LAST_RESULTS
    nc = _build()
    in_maps = make_in_maps(hidden, encoder_output, We, be, Wh, bh, Wv)
    res = run_bass_kernel_spmd(
        nc, in_maps, list(range(N_CORES)),
        trace=bool(os.environ.get("BASS_TRACE")),
    )
    LAST_RESULTS = res
    raw = np.concatenate(
        [res.results[c]["out"] for c in range(N_CORES)], axis=0
    )  # [B_TOT, 2, S_TOT]: rows = (hi, lo) partial energies, WV_SCALE * E
    energy = raw[:, 0, :].astype(np.float64) + raw[:, 1, :].astype(np.float64) / 16.0
    logits = (0.5 / WV_SCALE) * energy
    logits -= logits.max(axis=1, keepdims=True)
    ex = np.exp(logits)
    attn = ex / ex.sum(axis=1, keepdims=True)
    return np.ascontiguousarray(
        attn.reshape(B_TOT, 1, S_TOT).astype(np.float32)
    )



# revision 48
# speedup vs baseline: 1.0167x; 1.0167x over previous
"""Trainium2 Bass kernel for the additive-attention module.

Reference math (single device):
    enc    = einsum('sbh,kh->sbk', encoder_output, We) + be     # [S,B,K]
    hid    = hidden @ Wh.T + bh                                 # [B,K]
    energy = sigmoid(enc + hid[None]) @ Wv.T + bv               # [S,B,1]
    attn   = softmax(energy, axis=0)                            # over S
    out    = attn.transpose(1, 2, 0)                            # [B,1,S]

Device strategy (8 NeuronCores, data-parallel over batch):
  * Each core gets 8 of the 64 batches; weights replicated.
  * encoder_output is cast to fp8e4m3 on the host and laid out so each
    (s-block, batch) tile [128, 4, 1024] is a single contiguous 512 KB
    region in HBM - DMA streams it at full bandwidth.
  * sigmoid(x) = (1 + tanh(x/2))/2, and softmax is invariant to the
    affine constants, so the device computes
        E[s,b] = sum_k Wv[k] * tanh(0.5*enc_raw + hidb)
    (hidb = 0.5*(hidden @ Wh.T + bh + be), computed on host - 17 MFLOP)
    and the host finishes with softmax(0.5 * E) in float64.
  * enc matmuls run as fp8e4 DoubleRow; We host-scaled by 32 for fp8
    range, compensated in the activation input scale.
  * The PE is the bottleneck and runs at its structural floor: a warm
    (2.4 GHz, HAM K=8/8) fp8 DoubleRow matmul with free dim 512 costs
    exactly 512 cycles back-to-back, so 512 enc + 128 reduce matmuls =
    136.5us/core.  Everything else is engineered to keep the PE stream
    stall-free and the bookends (startup DMA, drain) off that path.
  * The tanh work (16.8M elems/core) is split across the Scalar engine
    (exact tanh) and the Vector engine (odd-quintic custom DVE op,
    u*(1 + u^2*(c3 + c5*u^2)), u = A*z, rms err 0.0045), greedily
    balanced by estimated per-tile cost.  Both write fp8 into shared
    [128, 2, 1024] sig tiles.
  * Iterations run in kc-major superblocks of 3 so each (kc, hc)
    stationary serves 6 consecutive matmuls; tile_legalize's per-matmul
    LDWEIGHTS are deduplicated post-build (see _dedupe_ldweights).
  * The Wv reduction over k rides the PE as fp8 DoubleRow with a
    2-column stationary operand (hi/lo pair: fp8(Wv*256) and its
    residual*16; the host recombines r0 + r1/16).  Reduces trail the
    enc stream by 2 superblocks so their sig semaphores are satisfied
    before the in-order PE reaches them.
  * Each iteration t directs its hi/lo result to partition rows
    (2t, 2t+1) of a persistent PSUM bank per s-half; one [64, 512]
    PSUM->SBUF copy + one scatter-DMA per s-half at the very end.
  * Startup: the PE clock idles at 1.2 GHz until ~3.4us of sustained
    activity (HAM); 28 dummy matmuls over a memset scratch burn the
    initial DMA wait so the real stream starts at 2.4 GHz.  WeT and
    iteration 0 lead the gpsimd SWDGE queue (the only ~350 GB/s path).
"""

import os
import numpy as np

import concourse.bass as bass
import concourse.mybir as mybir
import concourse.tile as tile
from concourse import bacc
from concourse.bass_utils import run_bass_kernel_spmd

S_TOT = 4096
B_TOT = 64
H = 512
N_CORES = 8
BPC = B_TOT // N_CORES  # batches per core
P = 128
KC = H // P  # 4 contraction / output chunks
SH = 1024    # s-chunk processed per activation tile
NMM = 512    # matmul moving free dim
NBLK = S_TOT // SH

F32 = mybir.dt.float32
F8 = mybir.dt.float8e4
WE_SCALE = 32.0
WV_SCALE = 256.0

# Odd-quintic tanh fit  t(z) = u*(1 + u^2*(QC3 + QC5*u^2)), u = QA*z,
# least-squares on the actual pre-activation distribution (|z| <= 2.3).
QA = 0.98079
QC3 = -0.25261
QC5 = 0.029919

# Results of the most recent device run (for the local test harness only).
LAST_RESULTS = None

_BUILD_CACHE = {}
_PWL_OP = None


def _dedupe_ldweights(nc):
    """Remove redundant InstLdweights after tile legalization.

    tile_legalize splits every InstMatmult into LDWEIGHTS + non-self-
    loading MATMUL.  The LDWEIGHTS serializes with the matmul on the PE
    (256 cycles each for a [128,2,128] fp8 DoubleRow stationary), so a
    stream of matmuls re-using one stationary pays 2x PE time for no
    reason.  A duplicate LDWEIGHTS (same operand AP as the previous one
    on the PE stream, no intervening self-loading matmul, and no
    attached semaphore waits/updates) is dead weight - delete it.
    Must run BEFORE nc.compile(), which moves matmul waits onto the
    preceding LDWEIGHTS.
    """
    import json
    for fn in nc.m.functions:
        for bb in fn.blocks:
            last_key = None
            to_remove = []
            for inst in bb.instructions:
                tn = type(inst).__name__
                if getattr(inst, "engine", None) != mybir.EngineType.PE:
                    continue
                if tn == "InstLdweights":
                    d = json.loads(mybir.instruction_to_pretty_json_string(inst))
                    for k in ("name", "sync_info", "debug", "bass_addl_debug"):
                        d.pop(k, None)
                    key = json.dumps(d, sort_keys=True)
                    si = inst.sync_info
                    clean = si is None or (
                        len(si.on_wait) == 0 and len(si.on_update) == 0
                    )
                    if clean and key == last_key:
                        to_remove.append(inst)
                    else:
                        last_key = key
                elif tn == "InstMatmult":
                    if inst.ldweights is not False:
                        last_key = None
            for inst in to_remove:
                bb.instructions.remove(inst)


def _register_dve_tanh_op():
    """Register the custom odd-quintic DVE op (idempotent).

    body: u = Src0*C1 + C0; w = u*u; out = u*(One + w*(C2 + C3*w))
    C0 = per-partition bias (A*hidb), C1 = input scale, C2 = c3,
    C3 (spilled to in1 as a [P,1] broadcast) = c5.  Exactly 8 ALU stages.
    """
    global _PWL_OP
    if _PWL_OP is not None:
        return _PWL_OP
    import concourse.dve_ops as dve_ops_mod
    from concourse.dve_ops import DveOp
    from concourse.dve_spec import (
        Spec, Src0, C0, C1, C2, C3, One, sq, lower,
        _spill_c3_to_src1, _has_src1,
    )
    from concourse.dve_uop import DveOpSpec

    name = "TANH_QUINTIC_ANT"
    for op in dve_ops_mod.OPS:
        if op.name == name:
            _PWL_OP = op
            return op

    u = Src0 * C1 + C0
    w = sq(u)
    body = _spill_c3_to_src1(u * (One + w * (C2 + C3 * w)))

    def _ref(in0, in1, s0, s1, imm2):
        uu = in0.astype(np.float32) * s1 + s0
        ww = uu * uu
        return uu * (1.0 + ww * (imm2 + in1 * ww))

    spec = Spec(body=body, reference=_ref)
    opcode = dve_ops_mod._CUSTOM_DVE_ROW_BASE + len(dve_ops_mod.OPS)
    shas = {}
    for ver in ("v3", "v4"):
        tmp = DveOpSpec(
            name=name, opcode=opcode, uops=lower(spec, ver=ver),
            rd1_en=_has_src1(spec),
        )
        shas[ver] = tmp.sha(ver)
    op = DveOp(name, spec, subdim=False, uops_sha=shas)
    dve_ops_mod.OPS.append(op)
    dve_ops_mod._SUB_OPCODE_FOR_NAME[name] = opcode
    dve_ops_mod.CUSTOM_DVE_SPECS[name] = spec
    _PWL_OP = op
    return op


def _build(s_tot=S_TOT, bpc=BPC, n_cores=N_CORES):
    key = (s_tot, bpc, n_cores)
    if key in _BUILD_CACHE:
        return _BUILD_CACHE[key]
    pwl_op = _register_dve_tanh_op()

    nc = bacc.Bacc(
        "TRN2", target_bir_lowering=False, debug=False, num_devices=n_cores
    )
    nblk = s_tot // SH
    eo8 = nc.dram_tensor("eo8", [nblk, bpc, 2, P, KC, NMM], F8,
                         kind="ExternalInput")
    WeT = nc.dram_tensor("WeT", [P, KC, H], F8, kind="ExternalInput")
    # bias tensor, two variants per kc: [:, :, 0] = hidb (ACT tanh bias),
    # [:, :, 1] = QA*hidb (DVE quintic bias)
    hidb = nc.dram_tensor("hidb", [P, KC, 2, bpc], F32, kind="ExternalInput")
    Wvp = nc.dram_tensor("Wvp", [P, nblk * bpc, 2, 2, 64], F8,
                         kind="ExternalInput")
    c5t = nc.dram_tensor("c5t", [P, 1], F32, kind="ExternalInput")
    out = nc.dram_tensor("out", [bpc, 2, s_tot], F32, kind="ExternalOutput")
    niter = nblk * bpc

    nns = SH // NMM
    Tanh = mybir.ActivationFunctionType.Tanh
    DRm = mybir.MatmulPerfMode.DoubleRow
    # Superblock: T_SB iterations processed kc-major so each (kc, hc)
    # stationary serves T_SB*nns consecutive matmuls -> the dedupe pass
    # keeps one LDWEIGHTS per group (the LDW->matmul weight-load latency
    # bubble, ~150-200ns, then amortizes over 6 matmuls instead of 2).
    T_SB = 3
    # Reduces trail by LAG_SB superblocks so their sig semaphores are
    # satisfied before the in-order PE reaches them.
    LAG_SB = 2

    with tile.TileContext(nc) as tc:
        with (
            tc.tile_pool(name="weights", bufs=1) as wpool,
            tc.tile_pool(name="ebuf", bufs=32) as epool,
            tc.tile_pool(name="sig", bufs=40) as sigpool,
            tc.tile_pool(name="estage", bufs=4) as stpool,
            tc.tile_pool(name="enc", bufs=3, space="PSUM") as encpool,
            tc.tile_pool(name="epsum", bufs=2, space="PSUM") as enpool,
        ):
            # Startup: iteration 0's two halves ride the sync and scalar
            # HWDGE queues (parallel to everything); WeT leads the gpsimd
            # SWDGE queue (16-engine fan-out) followed by the eo stream.
            # WeT then iteration 0's halves lead the gpsimd SWDGE queue —
            # the only queue with full 16-engine fan-out (~350 GB/s once
            # ramped; the sync/scalar HWDGE queues crawl at ~25 GB/s).
            # WeT and iteration 0 are split into hc-pair pieces ordered so
            # the first matmul group (j=0, hc=0) gates on just 256 KB
            WeT_sb = wpool.tile([P, KC, H], F8, tag="WeT")
            ebufs = {}  # t -> [half0, half1]
            ebufs[0] = [epool.tile([P, KC, NMM], F8, tag="ebuf", name=f"eb0_{h}")
                        for h in range(2)]
            for hc in range(0, KC, 2):
                nc.gpsimd.dma_start(WeT_sb[:, hc:hc + 2, :],
                                    WeT.ap()[:, hc:hc + 2, :])
                for h in range(2):
                    nc.gpsimd.dma_start(ebufs[0][h][:, hc:hc + 2, :],
                                        eo8.ap()[0, 0, h, :, hc:hc + 2, :])
            hidb_sb = wpool.tile([P, KC, 2, bpc], F32, tag="hidb")
            nc.sync.dma_start(hidb_sb[:], hidb.ap())
            c5_sb = wpool.tile([P, 1], F32, tag="c5t")
            nc.sync.dma_start(c5_sb[:], c5t.ap())
            # Wvp (1 MB): first use is the first reduce (~LAG_SB superblocks
            # in); rides the fast SWDGE queue after iteration 2's triggers
            Wv_sb = wpool.tile([P, niter, 2, 2, 64], F8, tag="Wvp")

            # persistent hi/lo energy accumulators: one PSUM bank per
            # s-half; iteration t lands on partition rows (2t, 2t+1)
            eps = [
                enpool.tile([P, NMM], F32, tag="epsum", name=f"eps{ns}")
                for ns in range(nns)
            ]

            # HAM warm-up: the PE clock idles at 1.2 GHz and only doubles
            # after ~3.4us of sustained activity.  The first real matmul
            # can't start until WeT + iteration 0 land (~15us), so burn the
            # DMA wait on dependency-free dummy matmuls over a memset
            # scratch tile.  They write (start=True groups) into eps, which
            # iteration 0's real reduce erases with its own start=True.
            warm = wpool.tile([P, 2, NMM], F8, tag="warm")
            nc.vector.memset(warm[:], 0)
            for w in range(20):
                nc.tensor.matmul(
                    eps[w % 2][0:64, :], warm[:, :, 0:64], warm[:],
                    start=True, stop=True, perf_mode=DRm,
                )

            def emit_reduce(t, sigs, order=(0, 1)):
                for idx, j in enumerate(order):
                    for ns in range(nns):
                        nc.tensor.matmul(
                            eps[ns][0:64, :],
                            Wv_sb[:, t, j],
                            sigs[j][:, :, ns * NMM:(ns + 1) * NMM],
                            start=(t == 0 and idx == 0),
                            stop=(t == niter - 1 and idx == 1),
                            perf_mode=DRm,
                        )

            # greedy ACT/DVE balance by estimated per-tile cost (us)
            ACT_COST, DVE_COST = 1.125, 1.28
            eng_load = [0.0, 0.0]  # ACT, DVE

            def emit_act(sig, r, enc_t, kc, b):
                use_act = (eng_load[0] + ACT_COST) <= (eng_load[1] + DVE_COST)
                if use_act:
                    eng_load[0] += ACT_COST
                    nc.scalar.activation(
                        sig[:, r, :], enc_t[:], Tanh,
                        scale=0.5 / WE_SCALE,
                        bias=hidb_sb[:, kc, 0, b:b + 1],
                    )
                else:
                    eng_load[1] += DVE_COST
                    nc.vector._custom_dve(
                        pwl_op,
                        out=sig[:, r, :], in0=enc_t[:],
                        in1=c5_sb[:],
                        s0=hidb_sb[:, kc, 1, b:b + 1],
                        s1=QA * 0.5 / WE_SCALE,
                        imm2=QC3,
                    )

            # ramp-in: tiny first superblocks so the first matmuls only
            # gate on iteration 0's DMA, not three iterations' worth
            sbs = [[0], [1], [2]] + [
                list(range(t0, min(t0 + T_SB, niter)))
                for t0 in range(3, niter, T_SB)
            ]
            pend = []  # [(t, sigs, order), ...] awaiting reduce
            for sbi, ts in enumerate(sbs):
                # DMA triggers for this superblock (iteration 0 pre-issued)
                for t in ts:
                    if t == 0:
                        continue
                    blk, b = t // bpc, t % bpc
                    ebufs[t] = [
                        epool.tile([P, KC, NMM], F8, tag="ebuf",
                                   name=f"eb{t}_{h}")
                        for h in range(2)
                    ]
                    for h in range(2):
                        nc.gpsimd.dma_start(ebufs[t][h][:], eo8.ap()[blk, b, h])
                    if t == 2:
                        nc.gpsimd.dma_start(Wv_sb[:], Wvp.ap())
                due = len(sbs[sbi - LAG_SB]) if sbi >= LAG_SB else 0
                sigs_of = {t: {} for t in ts}
                if sbi == 0:
                    # iteration 0, DMA-paced: hc-major so the first 4
                    # matmuls only need the hc 0-1 pieces of WeT and eb0
                    t = ts[0]
                    for j in (0, 1):
                        encs = {
                            r: encpool.tile([P, SH], F32, tag="enc",
                                            name=f"enc{t}_{2 * j + r}")
                            for r in range(2)
                        }
                        sigs_of[t][j] = sigpool.tile(
                            [P, 2, SH], F8, tag="sig", name=f"sig{t}_{j}"
                        )
                        for hc in range(0, KC, 2):
                            for r in range(2):
                                kc = 2 * j + r
                                for ns in range(nns):
                                    s0 = ns * NMM
                                    nc.tensor.matmul(
                                        encs[r][:, s0:s0 + NMM],
                                        WeT_sb[:, hc:hc + 2, kc * P:(kc + 1) * P],
                                        ebufs[t][ns][:, hc:hc + 2, :],
                                        start=(hc == 0),
                                        stop=(hc == KC - 2),
                                        perf_mode=DRm,
                                    )
                        for r in range(2):
                            emit_act(sigs_of[t][j], r, encs[r], 2 * j + r,
                                     t % bpc)
                else:
                    # kc-major enc matmuls + activations as tiles complete;
                    # trailing reduces ride mid-superblock (after the j=0
                    # phases) so stationary switches stay spread out
                    for j in (0, 1):
                        for r in range(2):
                            kc = 2 * j + r
                            encs = {}
                            for t in ts:
                                encs[t] = encpool.tile(
                                    [P, SH], F32, tag="enc", name=f"enc{t}_{kc}"
                                )
                            for hc in range(0, KC, 2):
                                for t in ts:
                                    for ns in range(nns):
                                        s0 = ns * NMM
                                        nc.tensor.matmul(
                                            encs[t][:, s0:s0 + NMM],
                                            WeT_sb[:, hc:hc + 2, kc * P:(kc + 1) * P],
                                            ebufs[t][ns][:, hc:hc + 2, :],
                                            start=(hc == 0),
                                            stop=(hc == KC - 2),
                                            perf_mode=DRm,
                                        )
                            for t in ts:
                                b = t % bpc
                                if r == 0:
                                    sigs_of[t][j] = sigpool.tile(
                                        [P, 2, SH], F8, tag="sig",
                                        name=f"sig{t}_{j}"
                                    )
                                emit_act(sigs_of[t][j], r, encs[t], kc, b)
                for _ in range(due):
                    emit_reduce(*pend.pop(0))
                for t in ts:
                    order = (1, 0) if t == niter - 1 else (0, 1)
                    pend.append((t, sigs_of[t], order))
                    del ebufs[t]
            for p in pend:
                emit_reduce(*p)

            # drain: one copy + one scatter-DMA per s-half
            # out[b, h, blk*SH + ns*NMM + s] <- eps[ns][2*(blk*bpc+b)+h, s]
            out_r = out.ap().rearrange(
                "b h (blk ns s) -> ns blk b h s", blk=nblk, ns=nns
            )
            for ns in range(nns):
                stage = stpool.tile([2 * niter, NMM], F32, tag="estage")
                if ns == 0:
                    nc.vector.tensor_copy(stage[:], eps[ns][0:2 * niter, :])
                else:
                    nc.scalar.copy(stage[:], eps[ns][0:2 * niter, :])
                # parallel drain: each s-half DMAs via its own DGE queue
                if ns == 0:
                    nc.sync.dma_start(out_r[ns], stage[:])
                else:
                    nc.scalar.dma_start(out_r[ns], stage[:])

    _dedupe_ldweights(nc)
    nc.compile()
    _BUILD_CACHE[key] = nc
    return nc


def make_in_maps(hidden, encoder_output, We, be, Wh, bh, Wv):
    """Host-side sharding/layout prep. Returns per-core input dicts."""
    import ml_dtypes
    f8 = ml_dtypes.float8_e4m3fn
    eo = np.asarray(encoder_output, dtype=np.float32)
    hidden = np.asarray(hidden, dtype=np.float32)
    WeT = np.ascontiguousarray(
        (np.asarray(We, np.float32).T * WE_SCALE)
        .reshape(KC, P, H).transpose(1, 0, 2)
    ).astype(f8)  # [P, KC(hc), H(k)]

    # hidb = 0.5 * (hidden @ Wh.T + bh + be); kc 2,3 pre-scaled by QA
    hid_all = 0.5 * (
        hidden @ np.asarray(Wh, np.float32).T
        + np.asarray(bh, np.float32) + np.asarray(be, np.float32)
    )  # [B_TOT, H]

    # Wv stationary pairs for fp8 DoubleRow: [P, pair j, plane r, col]
    # col 0 = fp8(Wv*256) (hi), col 1 = fp8(16*(Wv*256 - hi)) (lo)
    wv = np.asarray(Wv, np.float32).reshape(-1) * WV_SCALE  # [H]
    wv_hi = wv.astype(f8).astype(np.float32)
    wv_lo = (wv - wv_hi) * 16.0
    niter = NBLK * BPC
    Wvp = np.zeros((P, niter, 2, 2, 64), np.float32)
    for t in range(niter):
        c = 2 * t  # column pair selects the PSUM output row pair
        for j in range(2):
            for r in range(2):
                kc = 2 * j + r
                Wvp[:, t, j, r, c] = wv_hi[kc * P:(kc + 1) * P]
                Wvp[:, t, j, r, c + 1] = wv_lo[kc * P:(kc + 1) * P]
    Wvp = Wvp.astype(f8)

    c5t = np.full((P, 1), QC5, np.float32)

    # eo8[b][blk, h, p, c, s] = eo[blk*SH + h*NMM + s, b, c*128 + p] as fp8
    eo_r = eo.reshape(NBLK, 2, NMM, B_TOT, KC, P).transpose(3, 0, 1, 5, 4, 2)
    eo8_all = np.ascontiguousarray(eo_r).astype(f8)  # [B, nblk, 2, P, KC, NMM]

    in_maps = []
    for c in range(N_CORES):
        b0 = c * BPC
        eo8_c = np.ascontiguousarray(
            eo8_all[b0:b0 + BPC].transpose(1, 0, 2, 3, 4, 5)
        )  # [nblk, BPC, 2, P, KC, NMM]
        hidb_c = hid_all[b0:b0 + BPC].T.reshape(KC, P, BPC).transpose(1, 0, 2)
        hidb_c = np.stack([hidb_c, QA * hidb_c], axis=2)  # [P, KC, 2, BPC]
        in_maps.append({
            "eo8": eo8_c,
            "WeT": WeT,
            "hidb": np.ascontiguousarray(hidb_c),
            "Wvp": Wvp,
            "c5t": c5t,
        })
    return in_maps


def kernel(hidden, encoder_output, each_size=None, We=None, be=None,
           Wh=None, bh=None, Wv=None, bv=None):
    global LAST_RESULTS
    nc = _build()
    in_maps = make_in_maps(hidden, encoder_output, We, be, Wh, bh, Wv)
    res = run_bass_kernel_spmd(
        nc, in_maps, list(range(N_CORES)),
        trace=bool(os.environ.get("BASS_TRACE")),
    )
    LAST_RESULTS = res
    raw = np.concatenate(
        [res.results[c]["out"] for c in range(N_CORES)], axis=0
    )  # [B_TOT, 2, S_TOT]: rows = (hi, lo) partial energies, WV_SCALE * E
    energy = raw[:, 0, :].astype(np.float64) + raw[:, 1, :].astype(np.float64) / 16.0
    logits = (0.5 / WV_SCALE) * energy
    logits -= logits.max(axis=1, keepdims=True)
    ex = np.exp(logits)
    attn = ex / ex.sum(axis=1, keepdims=True)
    return np.ascontiguousarray(
        attn.reshape(B_TOT, 1, S_TOT).astype(np.float32)
    )



# revision 51
# speedup vs baseline: 1.0371x; 1.0201x over previous
"""Trainium2 Bass kernel for the additive-attention module.

Reference math (single device):
    enc    = einsum('sbh,kh->sbk', encoder_output, We) + be     # [S,B,K]
    hid    = hidden @ Wh.T + bh                                 # [B,K]
    energy = sigmoid(enc + hid[None]) @ Wv.T + bv               # [S,B,1]
    attn   = softmax(energy, axis=0)                            # over S
    out    = attn.transpose(1, 2, 0)                            # [B,1,S]

Device strategy (8 NeuronCores, data-parallel over batch):
  * Each core gets 8 of the 64 batches; weights replicated.
  * encoder_output is cast to fp8e4m3 on the host and laid out so each
    (s-block, batch) tile [128, 4, 1024] is a single contiguous 512 KB
    region in HBM - DMA streams it at full bandwidth.
  * sigmoid(x) = (1 + tanh(x/2))/2, and softmax is invariant to the
    affine constants, so the device computes
        E[s,b] = sum_k Wv[k] * tanh(0.5*enc_raw + hidb)
    (hidb = 0.5*(hidden @ Wh.T + bh + be), computed on host - 17 MFLOP)
    and the host finishes with softmax(0.5 * E) in float64.
  * enc matmuls run as fp8e4 DoubleRow; We host-scaled by 32 for fp8
    range, compensated in the activation input scale.
  * The PE is the bottleneck and runs at its structural floor: a warm
    (2.4 GHz, HAM K=8/8) fp8 DoubleRow matmul with free dim 512 costs
    exactly 512 cycles back-to-back, so 512 enc + 128 reduce matmuls =
    136.5us/core.  Everything else is engineered to keep the PE stream
    stall-free and the bookends (startup DMA, drain) off that path.
  * The tanh work (16.8M elems/core) is split across the Scalar engine
    (exact tanh) and the Vector engine (odd-quintic custom DVE op,
    u*(1 + u^2*(c3 + c5*u^2)), u = A*z, rms err 0.0045), greedily
    balanced by estimated per-tile cost.  Both write fp8 into shared
    [128, 2, 1024] sig tiles.
  * Iterations run in kc-major superblocks of 3 so each (kc, hc)
    stationary serves 6 consecutive matmuls; tile_legalize's per-matmul
    LDWEIGHTS are deduplicated post-build (see _dedupe_ldweights).
  * The Wv reduction over k rides the PE as fp8 DoubleRow with a
    2-column stationary operand (hi/lo pair: fp8(Wv*256) and its
    residual*16; the host recombines r0 + r1/16).  Reduces trail the
    enc stream by 2 superblocks so their sig semaphores are satisfied
    before the in-order PE reaches them.
  * Each iteration t directs its hi/lo result to partition rows
    (2t, 2t+1) of a persistent PSUM bank per s-half; one [64, 512]
    PSUM->SBUF copy + one scatter-DMA per s-half at the very end.
  * Startup: the PE clock idles at 1.2 GHz until ~3.4us of sustained
    activity (HAM); 28 dummy matmuls over a memset scratch burn the
    initial DMA wait so the real stream starts at 2.4 GHz.  WeT and
    iteration 0 lead the gpsimd SWDGE queue (the only ~350 GB/s path).
"""

import os
import numpy as np

import concourse.bass as bass
import concourse.mybir as mybir
import concourse.tile as tile
from concourse import bacc
from concourse.bass_utils import run_bass_kernel_spmd

S_TOT = 4096
B_TOT = 64
H = 512
N_CORES = 8
BPC = B_TOT // N_CORES  # batches per core
P = 128
KC = H // P  # 4 contraction / output chunks
SH = 1024    # s-chunk processed per activation tile
NMM = 512    # matmul moving free dim
NBLK = S_TOT // SH

F32 = mybir.dt.float32
BF16 = mybir.dt.bfloat16
F8 = mybir.dt.float8e4
WE_SCALE = 32.0
WV_SCALE = 256.0

# Odd-quintic tanh fit  t(z) = u*(1 + u^2*(QC3 + QC5*u^2)), u = QA*z,
# least-squares on the actual pre-activation distribution (|z| <= 2.3).
QA = 0.98079
QC3 = -0.25261
QC5 = 0.029919

# Results of the most recent device run (for the local test harness only).
LAST_RESULTS = None

_BUILD_CACHE = {}
_PWL_OP = None


def _dedupe_ldweights(nc):
    """Remove redundant InstLdweights after tile legalization.

    tile_legalize splits every InstMatmult into LDWEIGHTS + non-self-
    loading MATMUL.  The LDWEIGHTS serializes with the matmul on the PE
    (256 cycles each for a [128,2,128] fp8 DoubleRow stationary), so a
    stream of matmuls re-using one stationary pays 2x PE time for no
    reason.  A duplicate LDWEIGHTS (same operand AP as the previous one
    on the PE stream, no intervening self-loading matmul, and no
    attached semaphore waits/updates) is dead weight - delete it.
    Must run BEFORE nc.compile(), which moves matmul waits onto the
    preceding LDWEIGHTS.
    """
    import json
    for fn in nc.m.functions:
        for bb in fn.blocks:
            last_key = None
            to_remove = []
            for inst in bb.instructions:
                tn = type(inst).__name__
                if getattr(inst, "engine", None) != mybir.EngineType.PE:
                    continue
                if tn == "InstLdweights":
                    d = json.loads(mybir.instruction_to_pretty_json_string(inst))
                    for k in ("name", "sync_info", "debug", "bass_addl_debug"):
                        d.pop(k, None)
                    key = json.dumps(d, sort_keys=True)
                    si = inst.sync_info
                    clean = si is None or (
                        len(si.on_wait) == 0 and len(si.on_update) == 0
                    )
                    if clean and key == last_key:
                        to_remove.append(inst)
                    else:
                        last_key = key
                elif tn == "InstMatmult":
                    if inst.ldweights is not False:
                        last_key = None
            for inst in to_remove:
                bb.instructions.remove(inst)


def _register_dve_tanh_op():
    """Register the custom odd-quintic DVE op (idempotent).

    body: u = Src0*C1 + C0; w = u*u; out = u*(One + w*(C2 + C3*w))
    C0 = per-partition bias (A*hidb), C1 = input scale, C2 = c3,
    C3 (spilled to in1 as a [P,1] broadcast) = c5.  Exactly 8 ALU stages.
    """
    global _PWL_OP
    if _PWL_OP is not None:
        return _PWL_OP
    import concourse.dve_ops as dve_ops_mod
    from concourse.dve_ops import DveOp
    from concourse.dve_spec import (
        Spec, Src0, C0, C1, C2, C3, One, sq, lower,
        _spill_c3_to_src1, _has_src1,
    )
    from concourse.dve_uop import DveOpSpec

    name = "TANH_QUINTIC_ANT"
    for op in dve_ops_mod.OPS:
        if op.name == name:
            _PWL_OP = op
            return op

    u = Src0 * C1 + C0
    w = sq(u)
    body = _spill_c3_to_src1(u * (One + w * (C2 + C3 * w)))

    def _ref(in0, in1, s0, s1, imm2):
        uu = in0.astype(np.float32) * s1 + s0
        ww = uu * uu
        return uu * (1.0 + ww * (imm2 + in1 * ww))

    spec = Spec(body=body, reference=_ref)
    opcode = dve_ops_mod._CUSTOM_DVE_ROW_BASE + len(dve_ops_mod.OPS)
    shas = {}
    for ver in ("v3", "v4"):
        tmp = DveOpSpec(
            name=name, opcode=opcode, uops=lower(spec, ver=ver),
            rd1_en=_has_src1(spec),
        )
        shas[ver] = tmp.sha(ver)
    op = DveOp(name, spec, subdim=False, uops_sha=shas)
    dve_ops_mod.OPS.append(op)
    dve_ops_mod._SUB_OPCODE_FOR_NAME[name] = opcode
    dve_ops_mod.CUSTOM_DVE_SPECS[name] = spec
    _PWL_OP = op
    return op


def _build(s_tot=S_TOT, bpc=BPC, n_cores=N_CORES):
    key = (s_tot, bpc, n_cores)
    if key in _BUILD_CACHE:
        return _BUILD_CACHE[key]
    pwl_op = _register_dve_tanh_op()

    nc = bacc.Bacc(
        "TRN2", target_bir_lowering=False, debug=False, num_devices=n_cores
    )
    nblk = s_tot // SH
    eo8 = nc.dram_tensor("eo8", [nblk, bpc, 2, P, KC, NMM], F8,
                         kind="ExternalInput")
    WeT = nc.dram_tensor("WeT", [P, KC, H], F8, kind="ExternalInput")
    # bias tensor, two variants per kc: [:, :, 0] = hidb (ACT tanh bias),
    # [:, :, 1] = QA*hidb (DVE quintic bias)
    hidb = nc.dram_tensor("hidb", [P, KC, 2, bpc], F32, kind="ExternalInput")
    Wvp = nc.dram_tensor("Wvp", [P, nblk * bpc, 2, 2, 64], F8,
                         kind="ExternalInput")
    c5t = nc.dram_tensor("c5t", [P, 1], F32, kind="ExternalInput")
    # energies stage + DMA as bf16: the drain queues are ~25 GB/s
    # bandwidth-bound, so halving the bytes halves the exposed epilogue
    # time; bf16's 2^-9 relative error on the hi/lo pair adds only ~2e-4
    # logit error against a ~8e-3 remaining budget
    out = nc.dram_tensor("out", [bpc, 2, s_tot], BF16, kind="ExternalOutput")
    niter = nblk * bpc

    nns = SH // NMM
    Tanh = mybir.ActivationFunctionType.Tanh
    DRm = mybir.MatmulPerfMode.DoubleRow
    # Superblock: T_SB iterations processed kc-major so each (kc, hc)
    # stationary serves T_SB*nns consecutive matmuls -> the dedupe pass
    # keeps one LDWEIGHTS per group (the LDW->matmul weight-load latency
    # bubble, ~150-200ns, then amortizes over 6 matmuls instead of 2).
    T_SB = 3
    # Reduces trail by LAG_SB superblocks so their sig semaphores are
    # satisfied before the in-order PE reaches them.
    LAG_SB = 2

    with tile.TileContext(nc) as tc:
        with (
            tc.tile_pool(name="weights", bufs=1) as wpool,
            tc.tile_pool(name="ebuf", bufs=32) as epool,
            tc.tile_pool(name="sig", bufs=40) as sigpool,
            tc.tile_pool(name="estage", bufs=4) as stpool,
            tc.tile_pool(name="enc", bufs=3, space="PSUM") as encpool,
            tc.tile_pool(name="epsum", bufs=2, space="PSUM") as enpool,
        ):
            # Startup: iteration 0's two halves ride the sync and scalar
            # HWDGE queues (parallel to everything); WeT leads the gpsimd
            # SWDGE queue (16-engine fan-out) followed by the eo stream.
            # WeT then iteration 0's halves lead the gpsimd SWDGE queue —
            # the only queue with full 16-engine fan-out (~350 GB/s once
            # ramped; the sync/scalar HWDGE queues crawl at ~25 GB/s).
            # WeT and iteration 0 are split into hc-pair pieces ordered so
            # the first matmul group (j=0, hc=0) gates on just 256 KB
            WeT_sb = wpool.tile([P, KC, H], F8, tag="WeT")
            ebufs = {}  # t -> [half0, half1]
            ebufs[0] = [epool.tile([P, KC, NMM], F8, tag="ebuf", name=f"eb0_{h}")
                        for h in range(2)]
            for hc in range(0, KC, 2):
                nc.gpsimd.dma_start(WeT_sb[:, hc:hc + 2, :],
                                    WeT.ap()[:, hc:hc + 2, :])
                for h in range(2):
                    nc.gpsimd.dma_start(ebufs[0][h][:, hc:hc + 2, :],
                                        eo8.ap()[0, 0, h, :, hc:hc + 2, :])
            hidb_sb = wpool.tile([P, KC, 2, bpc], F32, tag="hidb")
            nc.sync.dma_start(hidb_sb[:], hidb.ap())
            c5_sb = wpool.tile([P, 1], F32, tag="c5t")
            nc.sync.dma_start(c5_sb[:], c5t.ap())
            # Wvp (1 MB): first use is the first reduce (~LAG_SB superblocks
            # in); rides the fast SWDGE queue after iteration 2's triggers
            Wv_sb = wpool.tile([P, niter, 2, 2, 64], F8, tag="Wvp")

            # persistent hi/lo energy accumulators: one PSUM bank per
            # s-half; iteration t lands on partition rows (2t, 2t+1)
            eps = [
                enpool.tile([P, NMM], F32, tag="epsum", name=f"eps{ns}")
                for ns in range(nns)
            ]

            # HAM warm-up: the PE clock idles at 1.2 GHz and only doubles
            # after ~3.4us of sustained activity.  The first real matmul
            # can't start until WeT + iteration 0 land (~15us), so burn the
            # DMA wait on dependency-free dummy matmuls over a memset
            # scratch tile.  They write (start=True groups) into eps, which
            # iteration 0's real reduce erases with its own start=True.
            warm = wpool.tile([P, 2, NMM], F8, tag="warm")
            nc.vector.memset(warm[:], 0)
            for w in range(20):
                nc.tensor.matmul(
                    eps[w % 2][0:64, :], warm[:, :, 0:64], warm[:],
                    start=True, stop=True, perf_mode=DRm,
                )

            def emit_reduce(t, sigs, order=(0, 1)):
                for idx, j in enumerate(order):
                    for ns in range(nns):
                        nc.tensor.matmul(
                            eps[ns][0:64, :],
                            Wv_sb[:, t, j],
                            sigs[j][:, :, ns * NMM:(ns + 1) * NMM],
                            start=(t == 0 and idx == 0),
                            stop=(t == niter - 1 and idx == 1),
                            perf_mode=DRm,
                        )

            # greedy ACT/DVE balance by estimated per-tile cost (us)
            ACT_COST, DVE_COST = 1.125, 1.28
            eng_load = [0.0, 0.0]  # ACT, DVE

            def emit_act(sig, r, enc_t, kc, b):
                use_act = (eng_load[0] + ACT_COST) <= (eng_load[1] + DVE_COST)
                if use_act:
                    eng_load[0] += ACT_COST
                    nc.scalar.activation(
                        sig[:, r, :], enc_t[:], Tanh,
                        scale=0.5 / WE_SCALE,
                        bias=hidb_sb[:, kc, 0, b:b + 1],
                    )
                else:
                    eng_load[1] += DVE_COST
                    nc.vector._custom_dve(
                        pwl_op,
                        out=sig[:, r, :], in0=enc_t[:],
                        in1=c5_sb[:],
                        s0=hidb_sb[:, kc, 1, b:b + 1],
                        s1=QA * 0.5 / WE_SCALE,
                        imm2=QC3,
                    )

            # ramp-in: tiny first superblocks so the first matmuls only
            # gate on iteration 0's DMA, not three iterations' worth
            sbs = [[0], [1], [2]] + [
                list(range(t0, min(t0 + T_SB, niter)))
                for t0 in range(3, niter, T_SB)
            ]
            pend = []  # [(t, sigs, order), ...] awaiting reduce
            for sbi, ts in enumerate(sbs):
                # DMA triggers for this superblock (iteration 0 pre-issued)
                for t in ts:
                    if t == 0:
                        continue
                    blk, b = t // bpc, t % bpc
                    ebufs[t] = [
                        epool.tile([P, KC, NMM], F8, tag="ebuf",
                                   name=f"eb{t}_{h}")
                        for h in range(2)
                    ]
                    for h in range(2):
                        nc.gpsimd.dma_start(ebufs[t][h][:], eo8.ap()[blk, b, h])
                    if t == 2:
                        nc.gpsimd.dma_start(Wv_sb[:], Wvp.ap())
                due = len(sbs[sbi - LAG_SB]) if sbi >= LAG_SB else 0
                sigs_of = {t: {} for t in ts}
                if sbi == 0:
                    # iteration 0, DMA-paced: hc-major so the first 4
                    # matmuls only need the hc 0-1 pieces of WeT and eb0
                    t = ts[0]
                    for j in (0, 1):
                        encs = {
                            r: encpool.tile([P, SH], F32, tag="enc",
                                            name=f"enc{t}_{2 * j + r}")
                            for r in range(2)
                        }
                        sigs_of[t][j] = sigpool.tile(
                            [P, 2, SH], F8, tag="sig", name=f"sig{t}_{j}"
                        )
                        for hc in range(0, KC, 2):
                            for r in range(2):
                                kc = 2 * j + r
                                for ns in range(nns):
                                    s0 = ns * NMM
                                    nc.tensor.matmul(
                                        encs[r][:, s0:s0 + NMM],
                                        WeT_sb[:, hc:hc + 2, kc * P:(kc + 1) * P],
                                        ebufs[t][ns][:, hc:hc + 2, :],
                                        start=(hc == 0),
                                        stop=(hc == KC - 2),
                                        perf_mode=DRm,
                                    )
                        for r in range(2):
                            emit_act(sigs_of[t][j], r, encs[r], 2 * j + r,
                                     t % bpc)
                else:
                    # kc-major enc matmuls + activations as tiles complete;
                    # trailing reduces ride mid-superblock (after the j=0
                    # phases) so stationary switches stay spread out
                    for j in (0, 1):
                        for r in range(2):
                            kc = 2 * j + r
                            encs = {}
                            for t in ts:
                                encs[t] = encpool.tile(
                                    [P, SH], F32, tag="enc", name=f"enc{t}_{kc}"
                                )
                            for hc in range(0, KC, 2):
                                for t in ts:
                                    for ns in range(nns):
                                        s0 = ns * NMM
                                        nc.tensor.matmul(
                                            encs[t][:, s0:s0 + NMM],
                                            WeT_sb[:, hc:hc + 2, kc * P:(kc + 1) * P],
                                            ebufs[t][ns][:, hc:hc + 2, :],
                                            start=(hc == 0),
                                            stop=(hc == KC - 2),
                                            perf_mode=DRm,
                                        )
                            for t in ts:
                                b = t % bpc
                                if r == 0:
                                    sigs_of[t][j] = sigpool.tile(
                                        [P, 2, SH], F8, tag="sig",
                                        name=f"sig{t}_{j}"
                                    )
                                emit_act(sigs_of[t][j], r, encs[t], kc, b)
                for _ in range(due):
                    emit_reduce(*pend.pop(0))
                for t in ts:
                    order = (1, 0) if t == niter - 1 else (0, 1)
                    pend.append((t, sigs_of[t], order))
                    del ebufs[t]
            for p in pend:
                emit_reduce(*p)

            # drain: one copy + one scatter-DMA per s-half
            # out[b, h, blk*SH + ns*NMM + s] <- eps[ns][2*(blk*bpc+b)+h, s]
            out_r = out.ap().rearrange(
                "b h (blk ns s) -> ns blk b h s", blk=nblk, ns=nns
            )
            for ns in range(nns):
                stage = stpool.tile([2 * niter, NMM], BF16, tag="estage")
                if ns == 0:
                    nc.vector.tensor_copy(stage[:], eps[ns][0:2 * niter, :])
                else:
                    nc.scalar.copy(stage[:], eps[ns][0:2 * niter, :])
                # parallel drain: each s-half DMAs via its own DGE queue
                if ns == 0:
                    nc.sync.dma_start(out_r[ns], stage[:])
                else:
                    nc.scalar.dma_start(out_r[ns], stage[:])

    _dedupe_ldweights(nc)
    nc.compile()
    _BUILD_CACHE[key] = nc
    return nc


def make_in_maps(hidden, encoder_output, We, be, Wh, bh, Wv):
    """Host-side sharding/layout prep. Returns per-core input dicts."""
    import ml_dtypes
    f8 = ml_dtypes.float8_e4m3fn
    eo = np.asarray(encoder_output, dtype=np.float32)
    hidden = np.asarray(hidden, dtype=np.float32)
    WeT = np.ascontiguousarray(
        (np.asarray(We, np.float32).T * WE_SCALE)
        .reshape(KC, P, H).transpose(1, 0, 2)
    ).astype(f8)  # [P, KC(hc), H(k)]

    # hidb = 0.5 * (hidden @ Wh.T + bh + be); kc 2,3 pre-scaled by QA
    hid_all = 0.5 * (
        hidden @ np.asarray(Wh, np.float32).T
        + np.asarray(bh, np.float32) + np.asarray(be, np.float32)
    )  # [B_TOT, H]

    # Wv stationary pairs for fp8 DoubleRow: [P, pair j, plane r, col]
    # col 0 = fp8(Wv*256) (hi), col 1 = fp8(16*(Wv*256 - hi)) (lo)
    wv = np.asarray(Wv, np.float32).reshape(-1) * WV_SCALE  # [H]
    wv_hi = wv.astype(f8).astype(np.float32)
    wv_lo = (wv - wv_hi) * 16.0
    niter = NBLK * BPC
    Wvp = np.zeros((P, niter, 2, 2, 64), np.float32)
    for t in range(niter):
        c = 2 * t  # column pair selects the PSUM output row pair
        for j in range(2):
            for r in range(2):
                kc = 2 * j + r
                Wvp[:, t, j, r, c] = wv_hi[kc * P:(kc + 1) * P]
                Wvp[:, t, j, r, c + 1] = wv_lo[kc * P:(kc + 1) * P]
    Wvp = Wvp.astype(f8)

    c5t = np.full((P, 1), QC5, np.float32)

    # eo8[b][blk, h, p, c, s] = eo[blk*SH + h*NMM + s, b, c*128 + p] as fp8
    eo_r = eo.reshape(NBLK, 2, NMM, B_TOT, KC, P).transpose(3, 0, 1, 5, 4, 2)
    eo8_all = np.ascontiguousarray(eo_r).astype(f8)  # [B, nblk, 2, P, KC, NMM]

    in_maps = []
    for c in range(N_CORES):
        b0 = c * BPC
        eo8_c = np.ascontiguousarray(
            eo8_all[b0:b0 + BPC].transpose(1, 0, 2, 3, 4, 5)
        )  # [nblk, BPC, 2, P, KC, NMM]
        hidb_c = hid_all[b0:b0 + BPC].T.reshape(KC, P, BPC).transpose(1, 0, 2)
        hidb_c = np.stack([hidb_c, QA * hidb_c], axis=2)  # [P, KC, 2, BPC]
        in_maps.append({
            "eo8": eo8_c,
            "WeT": WeT,
            "hidb": np.ascontiguousarray(hidb_c),
            "Wvp": Wvp,
            "c5t": c5t,
        })
    return in_maps


def kernel(hidden, encoder_output, each_size=None, We=None, be=None,
           Wh=None, bh=None, Wv=None, bv=None):
    global LAST_RESULTS
    nc = _build()
    in_maps = make_in_maps(hidden, encoder_output, We, be, Wh, bh, Wv)
    res = run_bass_kernel_spmd(
        nc, in_maps, list(range(N_CORES)),
        trace=bool(os.environ.get("BASS_TRACE")),
    )
    LAST_RESULTS = res
    raw = np.concatenate(
        [res.results[c]["out"] for c in range(N_CORES)], axis=0
    )  # [B_TOT, 2, S_TOT]: rows = (hi, lo) partial energies, WV_SCALE * E
    energy = raw[:, 0, :].astype(np.float64) + raw[:, 1, :].astype(np.float64) / 16.0
    logits = (0.5 / WV_SCALE) * energy
    logits -= logits.max(axis=1, keepdims=True)
    ex = np.exp(logits)
    attn = ex / ex.sum(axis=1, keepdims=True)
    return np.ascontiguousarray(
        attn.reshape(B_TOT, 1, S_TOT).astype(np.float32)
    )

